# revision 48
# baseline (speedup 1.0000x reference)
"""Trainium2 Bass kernel for the GNN interaction layer (e3nn-style message passing).

Strategy: partition edges across 8 cores by receiver shard (2500 nodes/core), so
scatter-add is core-local. Within a core, edges are grouped by 128-node receiver
block; scatter-add is a one-hot matmul on the PE accumulating in PSUM. The
one-hot matrices are precomputed host-side (they are just an index encoding)
and streamed in by DMA, keeping the vector engines free for the gate products.
Sender-feature gather uses one dma_gather per node block from a device-computed
f32 h table in DRAM (256B rows). All hot matmuls (radial MLP, L4 mix, scatter,
linear_down) run in bf16 (f32 PSUM accumulation). Spherical harmonics for all
edges are computed in a prologue vectorized across every chunk at once
([128, 360]-shaped ops instead of [128, 8]). The per-irrep linear_down is fused
per block (PE transposes + mix matmuls); the host reassembles the final
[20000, 576] output from feature-major per-core tiles.
"""
import math
import numpy as np
import ml_dtypes

from concourse import bacc, mybir, tile
from concourse.bass_utils import run_bass_kernel_spmd

F32 = mybir.dt.float32
BF16 = mybir.dt.bfloat16
I16 = mybir.dt.int16
I32 = mybir.dt.int32
AF = mybir.ActivationFunctionType
OP = mybir.AluOpType
NPBF16 = ml_dtypes.bfloat16

C = 64
R = 8
EPS = 0.5
N_NODES = 20000
N_EDGES = 320000
NCORES = 8
NS = N_NODES // NCORES          # nodes per core (2500)
NB = (NS + 127) // 128          # node blocks per core (20; last block 68 nodes)
S15 = math.sqrt(15.0)
S5H = 0.5 * math.sqrt(5.0)


# --------------------------------------------------------------------------
# host-side sharding / layout prep
# --------------------------------------------------------------------------

def _host_prep(vectors, node_feats, radial, senders, receivers):
    senders = np.asarray(senders)
    receivers = np.asarray(receivers)
    vectors = np.asarray(vectors, np.float32)
    radial = np.asarray(radial, np.float32)

    core = receivers // NS
    block = (receivers % NS) // 128
    gb = core * NB + block                       # global block id, 0..159
    order = np.argsort(gb, kind="stable")
    counts = np.bincount(gb, minlength=NCORES * NB)
    # per-block chunk count: max over cores, last block padded so the total
    # is a multiple of 8 (1024-edge super/call/MLP-group alignment)
    cb = counts.reshape(NCORES, NB).max(axis=0)
    chb = np.maximum(np.ceil(cb / 128.0).astype(int), 1)
    pad = (-chb.sum()) % 8
    chb[NB - 1] += pad
    CHB = tuple(int(x) for x in chb)
    cstart = np.concatenate([[0], np.cumsum(chb)])   # chunk offset per block
    NCH = int(chb.sum())
    TOT = NCH * 128                              # padded edges per core
    NSUP = TOT // 1024                           # 1024-edge SH super-groups

    # padded per-core edge arrays
    snd = np.zeros((NCORES, TOT), np.int16)
    rcl = np.full((NCORES, TOT), -1, np.int32)          # local recv in block
    vec = np.zeros((NCORES, TOT, 3), np.float32)
    vec[:, :, 0] = 1.0                                  # pad vectors: unit x
    rad = np.zeros((NCORES, TOT, R), np.float32)

    sorted_s = senders[order]
    sorted_r = receivers[order]
    sorted_v = vectors[order]
    sorted_rad = radial[order]
    starts = np.concatenate([[0], np.cumsum(counts)])
    for g in range(NCORES * NB):
        k, b = divmod(g, NB)
        n = counts[g]
        if n == 0:
            continue
        s0, d0 = starts[g], int(cstart[b]) * 128
        snd[k, d0:d0 + n] = sorted_s[s0:s0 + n].astype(np.int16)
        rcl[k, d0:d0 + n] = (sorted_r[s0:s0 + n] % NS) - b * 128
        vec[k, d0:d0 + n] = sorted_v[s0:s0 + n]
        rad[k, d0:d0 + n] = sorted_rad[s0:s0 + n]

    # h table permutation: h row for node n is perm[n], chosen so the
    # device's [128, kn, C] write tiles land contiguously per partition.
    HG = 1024
    perm = np.empty(N_NODES, np.int32)
    for t0 in range(0, N_NODES, HG):
        rows = min(HG, N_NODES - t0)
        kn = (rows + 127) // 128
        i = np.arange(rows)
        if rows % 128 == 0:
            perm[t0 + i] = t0 + (i % 128) * kn + i // 128
        else:
            perm[t0 + i] = t0 + i

    # dma_gather index layout: [128, TOT/16] int16; globally wrapped
    # (idx i at row i%16, col i//16), replicated on 8 q7 cores; any
    # 1024-aligned span is a valid per-call view.
    psnd = perm[snd.astype(np.int32)].astype(np.int16)
    wrapped = psnd.reshape(NCORES, TOT // 16, 16).transpose(0, 2, 1)
    idx = np.ascontiguousarray(np.tile(wrapped, (1, 8, 1)))

    # one-hot scatter matrices, host-precomputed: [128, TOT] bf16,
    # oh[p, ch*128 + n] = (rcl[ch*128+p] == n); pad edges (rcl=-1) all-zero.
    # (ch is the global chunk index; blocks span chb[b] chunks)
    oh = np.zeros((NCORES, 128, TOT), NPBF16)
    pos = np.arange(TOT)
    chs = pos // 128
    ps = pos % 128
    for k in range(NCORES):
        valid = rcl[k] >= 0
        oh[k, ps[valid], chs[valid] * 128 + rcl[k, valid]] = 1.0

    # vectors interleaved, all chunks: [128, NSUP*24], col su*24 + sub*3 + c
    vilT = vec.reshape(NCORES, NSUP, 8, 128, 3).transpose(0, 3, 1, 2, 4) \
              .reshape(NCORES, 128, NSUP * 24).copy()

    # radial transposed: [8, TOT] bf16
    radT = rad.transpose(0, 2, 1).astype(NPBF16)

    return dict(CHB=CHB, TOT=TOT, NSUP=NSUP, NCH=NCH, idx=idx, oh=oh,
                vilT=vilT, radT=radT)


def _scaled_weights(w_up, w1, w2, w3, w4, wd0, wd1, wd2):
    """Fold all constant scales into the weights; duplicate the MLP weights on
    both partition halves for the 2-group packed MLP. All bf16."""
    inv_sqrt_c = 1.0 / math.sqrt(C)
    w1s = (w1 / math.sqrt(R)).astype(np.float32)
    w2s = (w2 / math.sqrt(64.0)).astype(np.float32)
    w3s = (w3 / math.sqrt(64.0)).astype(np.float32)
    w4s = (w4 * (1.0 / math.sqrt(64.0)) * (1.0 / C)).astype(np.float32)
    w1d = np.zeros((128, 64), np.float32)
    w1d[0:R] = w1s
    w1d[64:64 + R] = w1s
    w2d = np.concatenate([w2s, w2s], axis=0)
    w3d = np.concatenate([w3s, w3s], axis=0)
    w4d = np.concatenate([w4s, w4s], axis=0)
    return dict(
        wup=(w_up * inv_sqrt_c).astype(NPBF16),
        w1d=w1d.astype(NPBF16), w2d=w2d.astype(NPBF16),
        w3d=w3d.astype(NPBF16), w4d=w4d.astype(NPBF16),
        wd0=(np.concatenate([wd0, wd0], 0) * EPS * inv_sqrt_c).astype(NPBF16),
        wd1=(np.concatenate([wd1, wd1], 0) * EPS * inv_sqrt_c).astype(NPBF16),
        wd2=(np.concatenate([wd2, wd2], 0) * EPS * inv_sqrt_c).astype(NPBF16),
    )


# --------------------------------------------------------------------------
# device program
# --------------------------------------------------------------------------

def _emit_sh_all(nc, shp, vilT_d, y1a, y2a, NSUP, NCH):
    """Spherical harmonics for ALL edges at once; ops shaped [128, NCH].
    y1a col = ch*3 + m, y2a col = ch*5 + m, bf16 (ch = chunk id)."""
    vil = shp.tile([128, NSUP * 24], F32, tag="vil")
    nc.sync.dma_start(out=vil[:], in_=vilT_d[:])
    sq = shp.tile([128, NSUP * 24], F32, tag="sq")
    nc.scalar.square(sq[:], vil[:])

    def comp(t, c):  # [128, NCH] view of component c (stride 3)
        return t[:, c::3]

    n2 = shp.tile([128, NCH], F32, tag="n2")
    nc.gpsimd.tensor_tensor(n2[:], comp(sq, 0), comp(sq, 1), OP.add)
    nc.gpsimd.tensor_tensor(n2[:], n2[:], comp(sq, 2), OP.add)
    rec = shp.tile([128, NCH], F32, tag="rec")
    nc.vector.reciprocal(rec[:], n2[:])
    r1 = shp.tile([128, NCH], F32, tag="r1")
    nc.scalar.activation(r1[:], rec[:], AF.Sqrt, scale=3.0)

    nc.vector.tensor_tensor(y1a[:, 0::3], comp(vil, 0), r1[:], OP.mult)
    nc.gpsimd.tensor_tensor(y1a[:, 1::3], comp(vil, 1), r1[:], OP.mult)
    nc.vector.tensor_tensor(y1a[:, 2::3], comp(vil, 2), r1[:], OP.mult)

    rec15 = shp.tile([128, NCH], F32, tag="rec15")
    nc.gpsimd.tensor_scalar(rec15[:], rec[:], S15, None, OP.mult)
    rec5h = shp.tile([128, NCH], F32, tag="rec5h")
    nc.vector.tensor_scalar(rec5h[:], rec[:], S5H, None, OP.mult)
    rec15h = shp.tile([128, NCH], F32, tag="rec15h")
    nc.gpsimd.tensor_scalar(rec15h[:], rec15[:], 0.5, None, OP.mult)

    xy = shp.tile([128, NCH], F32, tag="xy")
    nc.gpsimd.tensor_tensor(xy[:], comp(vil, 0), comp(vil, 1), OP.mult)
    yz = shp.tile([128, NCH], F32, tag="yz")
    nc.vector.tensor_tensor(yz[:], comp(vil, 1), comp(vil, 2), OP.mult)
    xz = shp.tile([128, NCH], F32, tag="xz")
    nc.gpsimd.tensor_tensor(xz[:], comp(vil, 0), comp(vil, 2), OP.mult)
    z3 = shp.tile([128, NCH], F32, tag="z3")
    nc.vector.tensor_scalar(z3[:], comp(sq, 2), 3.0, None, OP.mult)
    zc = shp.tile([128, NCH], F32, tag="zc")
    nc.gpsimd.tensor_tensor(zc[:], z3[:], n2[:], OP.subtract)
    dd = shp.tile([128, NCH], F32, tag="dd")
    nc.vector.tensor_tensor(dd[:], comp(sq, 0), comp(sq, 1), OP.subtract)

    def y2w(m, a, b, eng):
        out = y2a[:, 2 * m::10].unsqueeze(-1).broadcast_to((128, NCH, 2))
        out = y2a[:].rearrange("p (ch m) -> p ch m", m=10)[:, :, 2 * m:2 * m + 2]
        av = a[:].unsqueeze(-1).broadcast_to((128, NCH, 2))
        bv = b[:].unsqueeze(-1).broadcast_to((128, NCH, 2))
        eng.tensor_tensor(out, av, bv, OP.mult)

    y2w(0, xy, rec15, nc.gpsimd)
    y2w(1, yz, rec15, nc.vector)
    y2w(2, zc, rec5h, nc.gpsimd)
    y2w(3, xz, rec15, nc.vector)
    y2w(4, dd, rec15h, nc.gpsimd)


def _silu(nc, out, ps):
    if not SIM_SILU:
        nc.scalar.activation(out[:], ps[:], AF.Silu)
    else:
        nc.scalar.activation(out[:], ps[:], AF.Sigmoid)
        nc.vector.tensor_tensor(out[:], out[:], ps[:], OP.mult)


def _emit_mlp_pair(nc, apool, a3p, psm, pair, radsb, w1d, w2d, w3d):
    """MLP layers 1-3 for groups 2*pair (partitions 0-63) and 2*pair+1
    (partitions 64-127), packed via tile_position. bf16. Returns a3 [128, 512]."""
    ge, go = 2 * pair, 2 * pair + 1
    ps1 = psm.tile([128, 512], F32, tag="mlp")
    nc.tensor.matmul(ps1[0:64], w1d[0:R],
                     radsb[0:R, ge * 512:(ge + 1) * 512], start=True,
                     stop=True, tile_position=(0, 0))
    nc.tensor.matmul(ps1[64:128], w1d[64:64 + R],
                     radsb[64:64 + R, go * 512:(go + 1) * 512], start=True,
                     stop=True, tile_position=(64, 64))
    a1 = apool.tile([128, 512], BF16, tag="a1")
    _silu(nc, a1, ps1)

    ps2 = psm.tile([128, 512], F32, tag="mlp")
    nc.tensor.matmul(ps2[0:64], w2d[0:64], a1[0:64], start=True, stop=True,
                     tile_position=(0, 0))
    nc.tensor.matmul(ps2[64:128], w2d[64:128], a1[64:128], start=True,
                     stop=True, tile_position=(64, 64))
    a2 = apool.tile([128, 512], BF16, tag="a2")
    _silu(nc, a2, ps2)

    ps3 = psm.tile([128, 512], F32, tag="mlp")
    nc.tensor.matmul(ps3[0:64], w3d[0:64], a2[0:64], start=True, stop=True,
                     tile_position=(0, 0))
    nc.tensor.matmul(ps3[64:128], w3d[64:128], a2[64:128], start=True,
                     stop=True, tile_position=(64, 64))
    a3 = a3p.tile([128, 512], BF16, tag="a3")
    _silu(nc, a3, ps3)
    return a3


def _build(CHB, time_loops=1):
    NCH = sum(CHB)
    TOT = NCH * 128
    NG = TOT // 512
    NSUP = TOT // 1024
    cstart = [0]
    for c in CHB:
        cstart.append(cstart[-1] + c)
    assert TOT % 1024 == 0, (CHB, TOT)

    nc = bacc.Bacc(None, target_bir_lowering=False, debug=False,
                   dynamic_dma_scratch_size=65536)

    nfT_d = nc.dram_tensor("nfT", [C, N_NODES], BF16, kind="ExternalInput")
    wup_d = nc.dram_tensor("wup", [C, C], BF16, kind="ExternalInput")
    w1_d = nc.dram_tensor("w1d", [128, 64], BF16, kind="ExternalInput")
    w2_d = nc.dram_tensor("w2d", [128, 64], BF16, kind="ExternalInput")
    w3_d = nc.dram_tensor("w3d", [128, 64], BF16, kind="ExternalInput")
    w4_d = nc.dram_tensor("w4d", [128, 3 * C], BF16, kind="ExternalInput")
    wd_d = [nc.dram_tensor(f"wd{i}", [2 * C, C], BF16, kind="ExternalInput")
            for i in range(3)]
    idx_d = nc.dram_tensor("idx", [128, TOT // 16], I16, kind="ExternalInput")
    oh_d = nc.dram_tensor("oh", [128, TOT], BF16, kind="ExternalInput")
    vilT_d = nc.dram_tensor("vilT", [128, NSUP * 24], F32, kind="ExternalInput")
    radT_d = nc.dram_tensor("radT", [R, TOT], BF16, kind="ExternalInput")

    h_d = nc.dram_tensor("h", [N_NODES, C], F32)
    out_d = nc.dram_tensor("outp", [NB, C, 9, 128], BF16,
                           kind="ExternalOutput")

    with tile.TileContext(nc) as tc:
        with tc.tile_pool(name="const", bufs=1) as cpool:
            # SH result arrays (persistent, bf16); y1a col = ch*3+m,
            # y2a col = ch*10 + m*2 + r (value duplicated at r=0,1 so the
            # m2 product can run in the DVE 2x mode with a packed last dim)
            y1a = cpool.tile([128, 3 * NCH], BF16, tag="y1a")
            y2a = cpool.tile([128, 10 * NCH], BF16, tag="y2a")

            # ---- SH prologue first: its vil DMA leads the SP stream ----
            with tc.tile_pool(name="shp", bufs=1) as shp:
                _emit_sh_all(nc, shp, vilT_d, y1a, y2a, NSUP, NCH)

            wup = cpool.tile([C, C], BF16)
            nc.sync.dma_start(out=wup[:], in_=wup_d[:])

            # early DMAs for the main-loop critical path: gather indices and
            # the first radial chunk (first MLP pairs start during phase 1)
            idxt = cpool.tile([128, TOT // 16], I16)
            nc.sync.dma_start(out=idxt[:], in_=idx_d[:])
            radsb = cpool.tile([128, TOT], BF16, tag="radsb")
            c1_0 = TOT // 8
            nc.sync.dma_start(out=radsb[0:R, 0:c1_0], in_=radT_d[:, 0:c1_0])
            nc.sync.dma_start(out=radsb[64:64 + R, 0:c1_0],
                              in_=radT_d[:, 0:c1_0])

            # warm the Silu activation table while Act is idle
            actw = cpool.tile([1, 8], F32, tag="actw")
            nc.scalar.activation(actw[:], wup[0:1, 0:8],
                                 AF.Sigmoid if SIM_SILU else AF.Silu)

            # MLP weights: needed by pairs that overlap phase 1
            w1d = cpool.tile([128, 64], BF16, tag="w1d")
            nc.sync.dma_start(out=w1d[:], in_=w1_d[:])
            w2d = cpool.tile([128, 64], BF16, tag="w2d")
            nc.sync.dma_start(out=w2d[:], in_=w2_d[:])
            w3d = cpool.tile([128, 64], BF16, tag="w3d")
            nc.sync.dma_start(out=w3d[:], in_=w3_d[:])
            w4d = cpool.tile([128, 3 * C], BF16, tag="w4d")
            nc.sync.dma_start(out=w4d[:], in_=w4_d[:])
            wd = []
            for i in range(3):
                t = cpool.tile([2 * C, C], BF16, tag=f"wd{i}")
                wd.append(t)

            with tc.tile_pool(name="sb", bufs=4) as sbp, \
                 tc.tile_pool(name="ohp", bufs=2) as ohp, \
                 tc.tile_pool(name="ap", bufs=2) as apool, \
                 tc.tile_pool(name="a3p", bufs=6) as a3p, \
                 tc.tile_pool(name="msgp", bufs=4) as msgp, \
                 tc.tile_pool(name="wrp", bufs=2) as wrp, \
                 tc.tile_pool(name="psm", bufs=2, space="PSUM") as psm, \
                 tc.tile_pool(name="psx", bufs=2, space="PSUM") as psx, \
                 tc.tile_pool(name="psa", bufs=2, space="PSUM") as psa:

                # ---- phase 1: h = nf @ wup (nfT pool scoped) ----
                SLAB = 2048
                GRP = 1024
                with tc.tile_pool(name="nfT", bufs=2) as nfpool, \
                     tc.tile_pool(name="hsb", bufs=3) as hsb:
                    nft_tiles = {}

                    def slab_load(s0):
                        scols = min(SLAB, N_NODES - s0)
                        nft = nfpool.tile([C, SLAB], BF16, tag="nft",
                                          name=f"nft{s0}")
                        nc.sync.dma_start(out=nft[:, 0:scols],
                                          in_=nfT_d[:, s0:s0 + scols])
                        nft_tiles[s0] = nft

                    slab_load(0)
                    for s0 in range(0, N_NODES, SLAB):
                        scols = min(SLAB, N_NODES - s0)
                        if s0 + SLAB < N_NODES:
                            slab_load(s0 + SLAB)
                        nft = nft_tiles.pop(s0)
                        for t0 in range(s0, s0 + scols, GRP):
                            kn = min(GRP // 128,
                                     (s0 + scols - t0 + 127) // 128)
                            hb = hsb.tile([128, GRP // 128, C], F32,
                                          tag="hsb", name=f"hb{t0}")
                            ps = psm.tile([128, GRP // 128, C], F32,
                                          tag="mlp", name=f"hps{t0}")
                            for k in range(kn):
                                f0 = t0 + k * 128
                                nsz = min(128, s0 + scols - f0)
                                nc.tensor.matmul(
                                    ps[:nsz, k, :],
                                    nft[:, f0 - s0:f0 - s0 + nsz],
                                    wup[:], start=True, stop=True,
                                    skip_group_check=True)
                            rows = min(GRP, s0 + scols - t0)
                            if rows % 128 == 0:
                                nc.scalar.copy(
                                    hb[:, 0:kn, :]
                                    .rearrange("p k c -> p (k c)"),
                                    ps[:, 0:kn, :]
                                    .rearrange("p k c -> p (k c)"))
                            else:
                                for k in range(kn):
                                    f0 = t0 + k * 128
                                    nsz = min(128, s0 + scols - f0)
                                    nc.scalar.copy(hb[:nsz, k, :],
                                                   ps[:nsz, k, :])
                            if rows % 128 == 0:
                                hv = h_d[t0:t0 + rows].rearrange(
                                    "(p k) c -> p k c", k=kn)
                                nc.sync.dma_start(out=hv, in_=hb[:, 0:kn, :])
                            else:
                                for k in range(kn):
                                    f0 = t0 + k * 128
                                    nsz = min(128, s0 + scols - f0)
                                    nc.sync.dma_start(
                                        out=h_d[f0:f0 + nsz],
                                        in_=hb[:nsz, k, :])

                # late const DMAs (not needed until main loop)
                for i in range(3):
                    nc.sync.dma_start(out=wd[i][:], in_=wd_d[i][:])
                for qq in range(1, 8):
                    c0, c1 = qq * TOT // 8, (qq + 1) * TOT // 8
                    nc.sync.dma_start(out=radsb[0:R, c0:c1],
                                      in_=radT_d[:, c0:c1])
                    nc.sync.dma_start(out=radsb[64:64 + R, c0:c1],
                                      in_=radT_d[:, c0:c1])

                # ---- phase 2: main loop ----
                for _rep in range(time_loops):
                    next_pair = 0
                    a3_tiles = {}
                    blk_in = {}
                    NCALLS = TOT // 1024
                    call_tiles = {}
                    next_call = 0

                    def emit_gather(g):
                        sg = sbp.tile([128, 8, C], F32, tag="sblk",
                                      name=f"sg{g}")
                        nc.gpsimd.dma_gather(
                            sg[:], h_d[:], idxt[:, g * 64:(g + 1) * 64],
                            1024, 1024, C)
                        call_tiles[g] = sg

                    def prefetch(b, name):
                        e0 = cstart[b] * 128
                        ohb = ohp.tile([128, CHB[b] * 128], BF16, tag="ohb",
                                       name=f"ohb{name}")
                        nc.sync.dma_start(
                            out=ohb[:],
                            in_=oh_d[:, e0:e0 + CHB[b] * 128])
                        blk_in[b] = ohb

                    prefetch(0, "p0")
                    pend_wrap = None
                    wrap_state = None
                    wrap_im = 0

                    def wrap_start(b, pacc0, paccm):
                        # aggs rows (64-blocks in order) = agg cols im*64..
                        aggs = wrp.tile([128, 5, 128], BF16, tag="aggs",
                                        name=f"aggs{b}")
                        nc.scalar.copy(aggs[0:64, 0, :], pacc0[:])
                        nc.scalar.copy(
                            aggs[:, 1:5, :].rearrange("p q n -> p (q n)"),
                            paccm[:])
                        osb = wrp.tile([C, 9, 128], BF16, tag="osb",
                                       name=f"osb{b}")
                        return (b, aggs, osb)

                    def wrap_piece(state, im):
                        b, aggs, osb = state
                        irr = 0 if im == 0 else (1 if im < 4 else 2)
                        if im == 0:
                            asl = aggs[0:64, 0, :]
                            wsl = wd[irr][0:64]
                        else:
                            q = 1 + (im - 1) // 2
                            half = (im - 1) % 2
                            asl = aggs[64 * half:64 * half + 64, q, :]
                            wsl = wd[irr][64 * half:64 * half + 64]
                        ot = psm.tile([C, 128], F32, tag="mlp",
                                      name=f"ot{b}_{im}")
                        nc.tensor.matmul(ot[:], wsl, asl,
                                         start=True, stop=True)
                        nc.scalar.copy(osb[:, im, :], ot[:])

                    def wrap_end(state):
                        b, aggs, osb = state
                        nc.sync.dma_start(out=out_d[b], in_=osb[:])

                    for b in range(NB):
                        ohb = blk_in.pop(b)
                        if b + 1 < NB:
                            prefetch(b + 1, f"b{b}")

                        # feature-major scatter accumulators: rows = msg cols
                        # p0 = t0 (64); p1..p4 = msg cols 0:512 in 128-row tiles
                        pacc0 = psa.tile([64, 128], F32, tag="p0")
                        paccm = psa.tile([128, 512], F32, tag="pm")

                        CHb = CHB[b]

                        def emit_m2(rec):
                            j, kk, t_all2, msg2, ch = rec
                            for k in range(kk):
                                y2v = y2a[:, 10 * (ch + k):
                                          10 * (ch + k) + 10] \
                                    .rearrange("p (m q) -> p m q", q=2) \
                                    .unsqueeze(2) \
                                    .broadcast_to((128, 5, C // 2, 2))
                                t2v = t_all2[:, k, 2 * C:3 * C] \
                                    .rearrange("p (c q) -> p c q", q=2) \
                                    .unsqueeze(1) \
                                    .broadcast_to((128, 5, C // 2, 2))
                                m2v = msg2[:, k, 3 * C:8 * C].rearrange(
                                    "p (m c q) -> p m c q", m=5, q=2)
                                nc.vector.tensor_tensor(m2v, t2v, y2v,
                                                        OP.mult)

                        def emit_scatter(j, kk, t_all2, msg2, ch, pacc0=pacc0,
                                         paccm=paccm, ohb=ohb, CHb=CHb):
                            for k in range(kk):
                                jj = j + k
                                oh_j = ohb[:, jj * 128:(jj + 1) * 128]
                                st, sp = (jj == 0), (jj == CHb - 1)
                                nc.tensor.matmul(
                                    pacc0[:], t_all2[:, k, 0:C], oh_j,
                                    start=st, stop=sp)
                                for q in range(4):
                                    # start=True pends-zero the whole 2KB
                                    # bank; only the first matmul may set it
                                    nc.tensor.matmul(
                                        paccm[:, q * 128:(q + 1) * 128],
                                        msg2[:, k, q * 128:(q + 1) * 128],
                                        oh_j, start=(st and q == 0), stop=sp,
                                        skip_group_check=True)

                        pend_m2 = None
                        pend_sc = None
                        groups = []
                        jg = 0
                        while jg < CHb:
                            chg = cstart[b] + jg
                            kg = 1 if (chg % 2 == 1 or jg == CHb - 1) else 2
                            groups.append((jg, kg))
                            jg += kg
                        for j, kk in groups:
                            ch = cstart[b] + j
                            G, sub = divmod(ch, 4)
                            pair, parity = divmod(G, 2)

                            while next_pair * 8 <= ch + 17 and next_pair * 2 < NG:
                                a3_tiles[next_pair] = _emit_mlp_pair(
                                    nc, apool, a3p, psm, next_pair, radsb,
                                    w1d, w2d, w3d)
                                next_pair += 1
                            a3t = a3_tiles[pair]
                            while next_call * 8 <= ch + 17 and \
                                    next_call < NCALLS:
                                emit_gather(next_call)
                                next_call += 1
                            scall = call_tiles[ch // 8]
                            soff = ch % 8

                            # L4 for kk chunks: mix2[128e, kk*192] (bf16)
                            p0 = 64 * parity
                            mix2 = psx.tile([128, kk, 3 * C], F32, tag="mix",
                                            name=f"mix{ch}")
                            for k in range(kk):
                                nc.tensor.matmul(
                                    mix2[:, k, :],
                                    a3t[p0:p0 + 64,
                                        (sub + k) * 128:(sub + k + 1) * 128],
                                    w4d[p0:p0 + 64], start=True, stop=True,
                                    tile_position=(p0, 0))

                            # t_all2 = mix2 * s (DVE), bf16 out
                            t_all2 = msgp.tile([128, kk, 3 * C], BF16,
                                               tag="t_all", name=f"ta{ch}")
                            mixv = mix2[:].rearrange("p k (i c) -> p k i c", i=3)
                            sv = scall[:, soff:soff + kk, :].unsqueeze(2) \
                                .broadcast_to((128, kk, 3, C))
                            tv = t_all2[:].rearrange("p k (i c) -> p k i c", i=3)
                            nc.vector.tensor_tensor(tv, mixv, sv, OP.mult)

                            # msg2 = [m1 (192) | m2 (320)] x kk chunks
                            msg2 = msgp.tile([128, kk, 8 * C], BF16, tag="msg",
                                             name=f"mg{ch}")
                            y1v = y1a[:, 3 * ch:3 * ch + 3 * kk] \
                                .rearrange("p (k m) -> p k m", k=kk) \
                                .unsqueeze(-1).broadcast_to((128, kk, 3, C))
                            t1v = t_all2[:, :, C:2 * C].unsqueeze(2) \
                                .broadcast_to((128, kk, 3, C))
                            m1v = msg2[:, :, 0:3 * C].rearrange(
                                "p k (m c) -> p k m c", m=3)
                            nc.gpsimd.tensor_tensor(m1v, t1v, y1v, OP.mult)

                            # m2 lagged one group (keeps DVE off Pool's
                            # critical path); scatter lagged two
                            if pend_sc is not None:
                                emit_scatter(*pend_sc)
                                pend_sc = None
                            if pend_m2 is not None:
                                emit_m2(pend_m2)
                                pend_sc = pend_m2
                            pend_m2 = (j, kk, t_all2, msg2, ch)
                            # previous block's linear_down, spread over groups
                            if pend_wrap is not None:
                                wrap_state = wrap_start(*pend_wrap)
                                pend_wrap = None
                                wrap_im = 0
                            if wrap_state is not None and wrap_im < 9:
                                wrap_piece(wrap_state, wrap_im)
                                wrap_im += 1
                                if wrap_im == 9:
                                    wrap_end(wrap_state)
                                    wrap_state = None

                        if pend_sc is not None:
                            emit_scatter(*pend_sc)
                        if pend_m2 is not None:
                            emit_m2(pend_m2)
                            emit_scatter(*pend_m2)
                        while wrap_state is not None:
                            wrap_piece(wrap_state, wrap_im)
                            wrap_im += 1
                            if wrap_im == 9:
                                wrap_end(wrap_state)
                                wrap_state = None
                        pend_wrap = (b, pacc0, paccm)
                    wrap_state = wrap_start(*pend_wrap)
                    for wrap_im in range(9):
                        wrap_piece(wrap_state, wrap_im)
                    wrap_end(wrap_state)
                    wrap_state = None

    nc.compile()
    return nc


# --------------------------------------------------------------------------
# public entry point
# --------------------------------------------------------------------------

_CACHE = {}


def _get_program(CH, time_loops=1):
    key = (CH, time_loops)
    if key not in _CACHE:
        _CACHE[key] = _build(CH, time_loops)
    return _CACHE[key]


SIM_SILU = False  # CoreSim has no Silu; substitute sigmoid*x when set


def _make_in_maps(prep, sw, node_feats):
    nfT = np.ascontiguousarray(
        np.asarray(node_feats, np.float32).T).astype(NPBF16)
    maps = []
    for k in range(NCORES):
        maps.append({
            "nfT": nfT, "wup": sw["wup"], "w1d": sw["w1d"], "w2d": sw["w2d"],
            "w3d": sw["w3d"], "w4d": sw["w4d"], "wd0": sw["wd0"],
            "wd1": sw["wd1"], "wd2": sw["wd2"],
            "idx": prep["idx"][k], "oh": prep["oh"][k],
            "vilT": prep["vilT"][k], "radT": prep["radT"][k],
        })
    return maps


def _assemble(results):
    out = np.empty((N_NODES, 9 * C), np.float32)
    d = np.arange(C)
    for k in range(NCORES):
        ob = np.asarray(results[k]["outp"], np.float32)     # [NB, C, 9, 128]
        oc = ob.transpose(2, 1, 0, 3).reshape(9, C, NB * 128)
        tmp = np.empty((NB * 128, 9 * C), np.float32)
        tmp[:, 0:C] = oc[0].T
        for m in range(3):
            tmp[:, C + 3 * d + m] = oc[1 + m].T
        for m in range(5):
            tmp[:, 4 * C + 5 * d + m] = oc[4 + m].T
        for b in range(NB):
            bs = min(128, NS - b * 128)
            r0 = k * NS + b * 128
            out[r0:r0 + bs] = tmp[b * 128:b * 128 + bs]
    return out


def kernel(vectors, node_feats, radial_embedding, senders, receivers,
           w_up, mlp_w1, mlp_w2, mlp_w3, mlp_w4,
           w_down0, w_down1, w_down2):
    prep = _host_prep(vectors, node_feats, radial_embedding, senders, receivers)
    sw = _scaled_weights(w_up, mlp_w1, mlp_w2, mlp_w3, mlp_w4,
                         w_down0, w_down1, w_down2)
    nc = _get_program(prep["CHB"])
    in_maps = _make_in_maps(prep, sw, node_feats)
    res = run_bass_kernel_spmd(nc, in_maps, list(range(NCORES)))
    return _assemble(res.results)
